# revision 28
# baseline (speedup 1.0000x reference)
"""Causal multi-head attention (B=16, T=1024, E=1024, H=16, Dh=64) on 8 TRN2
NeuronCores.

Sharding: data-parallel over batch -- 2 batch elements per core, weights
replicated, no collectives. Host pre-transposes x and packs weights; each core
runs an identical Bass/Tile program on its shard.

Per-core dataflow (all in "transposed" orientation so no on-chip transposes
are ever needed):
  x^T [E,T] (host)   --matmul-->  Q^T,K^T [Dh,T] per head (head-pairs packed
                                  into 128 partitions; 1/sqrt(Dh) folded into
                                  the Q PSUM->SBUF copy)
                     --matmul-->  V [T,Dh] per head (+ ones column)
  S^T[tk,tq] = (K^T tile).T @ Q^T  per key-tile, causal tiles skipped; the
        diagonal 128x128 block gets -1e30 added by a const matmul
        (lhsT=I, rhs=-1e30*tril(,-1)) accumulated into the same PSUM group,
        so exp() lands masked with no vector-engine hop
  P^T = exp(S^T) on ScalarE (scores are O(1): no max subtraction needed)
  O'^T[65,tq] += (V'|1).T @ P^T   -- row 64 accumulates the softmax denom
  Y^T = O'^T[0:64] * bcast(1/denom)
  out[t,E] = Y^T.T @ Wo + bo

Engine assignment (ScalarE is the S->P->O critical hop, keep it exp-only):
  ScalarE: exp
  DVE:     all PSUM->SBUF copies (Q/K scaled copy, V, Y^T), den staging,
           reciprocal_approx_fast, normalize muls
  GpSimd:  partition_broadcast of 1/den (its only fast op; no PSUM access)
  PE:      matmuls incl. the diag-mask const matmul

Scheduling notes (hard-won on HW):
  - engine APs need partition base in {0,32,64,96}; partition_broadcast reads
    physical partition 0 and writes from partition 0 (base-64 dst broken);
    tensor_tensor wants equal bases when both operands are SBUF (a PSUM in0
    at base 0 with SBUF in1/out at base 64 is fine).
  - per-pair qT/kT/yT tiles: Tile deps are whole-tile, per-pair tiles stop
    S matmuls from false-waiting on the next pair's projection copies.
  - HAM unthrottles the PE (1.2 -> 2.4 GHz) only on a fully-busy 3.4us
    window, so each pair's Q/K projection matmuls are interleaved into the
    previous pair's attention stream as dense filler, and 8 of each batch's
    out-projection blocks are carried into the NEXT batch's pairs 6-7
    (which otherwise have no projection filler).
  - normalize chain (recip + bcast + mul) is drip-emitted between i-steps;
    the yT copy + den staging run eagerly so the O' PSUM banks free fast.
"""
import numpy as np
import ml_dtypes

import concourse.bass as bass
import concourse.mybir as mybir
import concourse.tile as tile
from concourse import bacc
from concourse.bass_utils import run_bass_kernel_spmd

B, T, E = 16, 1024, 1024
H, Dh = 16, 64
NCORES = 8
BL = B // NCORES          # batches per core
P = 128                   # partitions
ET = E // P               # 8 tiles along E / token / hd dims
HP = H // 2               # 8 head-pairs
BF = mybir.dt.bfloat16
F32 = mybir.dt.float32
AF = mybir.ActivationFunctionType
ALU = mybir.AluOpType

_CACHE = {}


def _pieces(i):
    """Column pieces of [128*i, 1024) that do not cross the 512 PSUM-bank
    boundary."""
    if i < 4:
        return [(128 * i, 512), (512, 1024)]
    return [(128 * i, 1024)]


def _build():
    nc = bacc.Bacc("TRN2", target_bir_lowering=False, debug=False,
                   num_devices=NCORES)

    xT = nc.dram_tensor("xT", [BL, E, T], BF, kind="ExternalInput").ap()
    wq = nc.dram_tensor("wq", [E, H * Dh], BF, kind="ExternalInput").ap()
    wk = nc.dram_tensor("wk", [E, H * Dh], BF, kind="ExternalInput").ap()
    wv = nc.dram_tensor("wv", [E, H * Dh], BF, kind="ExternalInput").ap()
    wo = nc.dram_tensor("wo", [H * Dh, E], BF, kind="ExternalInput").ap()
    borep = nc.dram_tensor("borep", [P, E], BF, kind="ExternalInput").ap()
    ident = nc.dram_tensor("ident", [P, P], BF, kind="ExternalInput").ap()
    uneg = nc.dram_tensor("uneg", [P, P], BF, kind="ExternalInput").ap()
    out = nc.dram_tensor("out", [BL, T, E], BF,
                         kind="ExternalOutput").ap()

    with tile.TileContext(nc) as tc:
        with (
            tc.tile_pool(name="consts", bufs=1) as cpool,
            tc.tile_pool(name="xp", bufs=1) as xpool,
            tc.tile_pool(name="qk", bufs=1) as qkpool,
            tc.tile_pool(name="vp2", bufs=2) as vpool,
            tc.tile_pool(name="vy", bufs=2) as vypool,
            tc.tile_pool(name="pt", bufs=4) as ptpool,
            tc.tile_pool(name="r1p", bufs=2) as r1pool,
            tc.tile_pool(name="rbp", bufs=2) as rbpool,
            tc.tile_pool(name="ob", bufs=2) as opool,
            tc.tile_pool(name="pso", bufs=2, space="PSUM") as pso,
            tc.tile_pool(name="psc", bufs=4, space="PSUM") as psc,
        ):
            # --- weights / consts; DMA order tracks the startup critical
            # path: V-projection needs Wv halves + xT halves first ---
            wva = cpool.tile([P, ET, 512], BF, tag="wva")
            wvb = cpool.tile([P, ET, 512], BF, tag="wvb")
            wq_sb = cpool.tile([P, ET, H * Dh], BF, tag="wq")
            wk_sb = cpool.tile([P, ET, H * Dh], BF, tag="wk")
            wo_sb = cpool.tile([P, ET, E], BF, tag="wo")
            id_sb = cpool.tile([P, P], BF, tag="ident")
            un_sb = cpool.tile([P, P], BF, tag="uneg")

            wv_r = wv.rearrange("(n p) c -> p n c", p=P)

            xT_tiles = {}
            v_tiles = {}

            def load_x(b, chunked=False):
                """xT halves (tokens 0:512 / 512:1024) for batch b. Chunked
                per E-tile so the first V-proj chain streams behind the DMA
                (Tile tracks subtile deps)."""
                xa = xpool.tile([P, ET, 512], BF, tag="xa", name=f"xa{b}")
                xb = xpool.tile([P, ET, 512], BF, tag="xb", name=f"xb{b}")
                xr = xT[b].rearrange("(n p) c -> p n c", p=P)
                if chunked:
                    for i in range(ET):
                        nc.sync.dma_start(wva[:, i, :], wv_r[:, i, 0:512])
                        nc.sync.dma_start(xa[:, i, :], xr[:, i, 0:512])
                    for i in range(ET):
                        nc.sync.dma_start(wvb[:, i, :], wv_r[:, i, 512:1024])
                        nc.sync.dma_start(xb[:, i, :], xr[:, i, 512:1024])
                else:
                    nc.sync.dma_start(xa[:], xr[:, :, 0:512])
                    nc.sync.dma_start(xb[:], xr[:, :, 512:1024])
                xT_tiles[b] = (xa, xb)
                return xa, xb, xr

            def load_v_tile(b):
                v_tiles[b] = vpool.tile([P, ET, H, Dh + 1], BF, tag="v",
                                        name=f"v{b}")
                nc.vector.memset(v_tiles[b][:, :, :, Dh], 1.0)

            def x_ap(b, t, cols):
                """lhsT slice of x^T for t-tile t (128 tokens) col range."""
                xa, xb = xT_tiles[b]
                if t < 4:
                    return xa[:, cols, 128 * t:128 * (t + 1)]
                return xb[:, cols, 128 * (t - 4):128 * (t - 3)]

            def v_blocks(b, order=None):
                """V-projection blocks; block (t, n2) computes v[:, t,
                8*n2:8*(n2+1), :]. Order tuned so the first blocks only
                need wva + xa."""
                if order is None:
                    order = [(t, n2) for n2 in range(2) for t in range(ET)]
                blocks = []
                for (t, n2) in order:
                    def vblk(t=t, n2=n2, b=b):
                        w_sb = wva if n2 == 0 else wvb
                        vp = psc.tile([P, 512], F32, tag="pc",
                                      name=f"vp{b}_{t}_{n2}")
                        for i in range(ET):
                            nc.tensor.matmul(
                                vp[:],
                                lhsT=x_ap(b, t, i),
                                rhs=w_sb[:, i, :],
                                start=(i == 0), stop=(i == ET - 1),
                            )
                        nc.scalar.activation(
                            v_tiles[b][:, t, 8 * n2:8 * (n2 + 1), 0:Dh],
                            vp[:].rearrange("p (h d) -> p h d", d=Dh),
                            AF.Copy)
                    blocks.append(vblk)
                return blocks

            # batch 0 startup: interleaved per-E-tile wv/x chunks
            load_x(0, chunked=True)
            load_v_tile(0)
            order0 = ([(t, 0) for t in range(4)] + [(t, 1) for t in range(4)]
                      + [(t, 0) for t in range(4, ET)]
                      + [(t, 1) for t in range(4, ET)])
            blocks0 = v_blocks(0, order0)
            for blk in blocks0[:8]:
                blk()
            nc.sync.dma_start(wq_sb[:], wq.rearrange("(n p) c -> p n c", p=P))
            nc.sync.dma_start(wk_sb[:], wk.rearrange("(n p) c -> p n c", p=P))
            nc.sync.dma_start(id_sb[:], ident)
            nc.sync.dma_start(un_sb[:], uneg)
            for blk in blocks0[8:]:
                blk()
            nc.sync.dma_start(wo_sb[:], wo.rearrange("(n p) c -> p n c", p=P))
            borep_sb = cpool.tile([P, E], BF, tag="bo")
            nc.sync.dma_start(borep_sb[:], borep)

            pending = []

            def drain(n):
                for _ in range(min(n, len(pending))):
                    pending.pop(0)()

            carry = []  # prev batch's deferred out-proj blocks
            qT_all, kT_all = {}, {}

            def ensure_qk(b):
                if b in qT_all:
                    return
                qT_all[b] = [qkpool.tile([P, T], BF, tag=f"q{pp}",
                                         name=f"q{b}_{pp}")
                             for pp in range(HP)]
                kT_all[b] = [qkpool.tile([P, T], BF, tag=f"k{pp}",
                                         name=f"k{b}_{pp}")
                             for pp in range(HP)]

            def proj_subblocks(b, pp):
                blocks = []
                for (w_sb, dstl, scale) in ((wq_sb, qT_all[b], 0.125),
                                            (wk_sb, kT_all[b], None)):
                    for n2 in range(2):
                        def blk(w_sb=w_sb, dstl=dstl, scale=scale,
                                n2=n2, pp=pp, b=b):
                            cs = slice(512 * n2, 512 * (n2 + 1))
                            xh = xT_tiles[b][n2]
                            pj = psc.tile([P, 512], F32, tag="pc",
                                          name=f"pj{b}_{pp}_{n2}")
                            for i in range(ET):
                                nc.tensor.matmul(
                                    pj[:],
                                    lhsT=w_sb[:, i,
                                              128 * pp:128 * (pp + 1)],
                                    rhs=xh[:, i, :],
                                    start=(i == 0), stop=(i == ET - 1),
                                )
                            if scale is None:
                                nc.vector.tensor_copy(dstl[pp][:, cs],
                                                      pj[:])
                            else:
                                nc.vector.tensor_scalar_mul(
                                    dstl[pp][:, cs], pj[:], scale)
                        blocks.append(blk)
                return blocks

            for b in range(BL):
                v_sb = v_tiles[b]
                ensure_qk(b)
                qT, kT = qT_all[b], kT_all[b]

                if b == 0:
                    for blk in proj_subblocks(0, 0):
                        blk()
                # (b>0: pair-0 projection was emitted at the end of b-1)

                # ---- per-pair attention ----
                yT = [vypool.tile([P, T], BF, tag=f"y{pp}", name=f"y{b}_{pp}")
                      for pp in range(HP)]

                for hp in range(HP):
                    if hp + 1 < HP:
                        pending[0:0] = proj_subblocks(b, hp + 1)
                    if hp == 6 and b + 1 < BL:
                        def ldnext(b=b):
                            load_x(b + 1)
                            load_v_tile(b + 1)
                        pending.append(ldnext)
                        pending.extend(v_blocks(b + 1))
                    if b + 1 == BL:
                        # prev batch's deferred out-proj is the only PE
                        # filler for the last two pairs: 8 blocks into
                        # pair 6 (drained ~2/i-step), 4 more into pair 7
                        if hp == 6:
                            pending.extend(carry[:8])
                        elif hp == 7:
                            pending.extend(carry[8:])
                            carry = []
                    ops = [pso.tile([P, 1024], F32, tag="op",
                                    name=f"op{b}_{hp}_{s}") for s in range(2)]
                    eager = (hp == HP - 1)
                    for i in range(ET):
                        pts = []
                        for sub in (0, 1):
                            pt = ptpool.tile([P, 1024], BF, tag="pt",
                                             name=f"pt{b}_{hp}_{i}_{sub}")
                            pts.append(pt)
                        for pidx, (a0, a1) in enumerate(_pieces(i)):
                            w = a1 - a0
                            sps = []
                            for sub in (0, 1):
                                po = Dh * sub
                                sp_ = psc.tile(
                                    [P, 512], F32, tag="pc",
                                    name=f"sp{b}_{hp}_{i}_{sub}_{a0}")
                                nc.tensor.matmul(
                                    sp_[:, 0:w],
                                    lhsT=kT[hp][po:po + Dh,
                                                128 * i:128 * (i + 1)],
                                    rhs=qT[hp][po:po + Dh, a0:a1],
                                    start=True, stop=(pidx != 0),
                                    skip_group_check=True,
                                )
                                sps.append(sp_)
                            if pidx == 0:
                                # diag 128x128 block: add -1e30 upper mask
                                for sub in (0, 1):
                                    nc.tensor.matmul(
                                        sps[sub][:, 0:P],
                                        lhsT=id_sb[:],
                                        rhs=un_sb[:],
                                        start=False, stop=True,
                                        skip_group_check=True,
                                    )
                            for sub in (0, 1):
                                nc.scalar.activation(pts[sub][:, a0:a1],
                                                     sps[sub][:, 0:w], AF.Exp)
                        for sub in (0, 1):
                            h = 2 * hp + sub
                            for (a0, a1) in _pieces(i):
                                nc.tensor.matmul(
                                    ops[sub][0:Dh + 1, a0:a1],
                                    lhsT=v_sb[:, i, h, :],
                                    rhs=pts[sub][:, a0:a1],
                                    start=(i == 0), stop=(i == ET - 1),
                                    skip_group_check=True,
                                )
                            if i == ET - 1:
                                po = Dh * sub
                                # eager: copy Y^T (DVE) + stage denom
                                # (ACT) so the O' PSUM banks free fast; the
                                # stage fills ScalarE's pair-boundary exp
                                # lull instead of delaying the next pair's
                                # exps behind a 1.1us yT copy
                                r1 = r1pool.tile([1, T], F32, tag="r1",
                                                 name=f"r1_{b}_{h}")
                                nc.vector.tensor_copy(
                                    yT[hp][po:po + Dh, :],
                                    ops[sub][0:Dh, :])
                                nc.scalar.activation(
                                    r1[0:1, :], ops[sub][Dh:Dh + 1, :],
                                    AF.Copy)

                                def recip(r1=r1):
                                    nc.vector.reciprocal_approx_fast(
                                        r1[0:1, :], r1[0:1, :])
                                holder = {}

                                def bcast(r1=r1, holder=holder, b=b, h=h):
                                    rb = rbpool.tile([P, T], F32, tag="rb",
                                                     name=f"rb_{b}_{h}")
                                    nc.gpsimd.partition_broadcast(rb[:],
                                                                  r1[0:1, :])
                                    holder['rb'] = rb

                                def nmul(hp=hp, po=po, holder=holder):
                                    ap = yT[hp]
                                    nc.vector.tensor_mul(
                                        ap[po:po + Dh, :], ap[po:po + Dh, :],
                                        holder['rb'][po:po + Dh, :])
                                if eager:
                                    recip(); bcast(); nmul()
                                else:
                                    pending.extend([recip, bcast, nmul])
                        drain(2)

                # emission order IS dependency order under Tile's tracer:
                # all normalize muls must be emitted before out-proj reads yT
                drain(len(pending))

                # ---- output projection + bias ----
                def outproj_blocks(b=b, yT=yT):
                    blocks = []
                    for t in range(ET):
                        for n2 in range(2):
                            def oblk(t=t, n2=n2, b=b, yT=yT):
                                cs = slice(512 * n2, 512 * (n2 + 1))
                                o2 = psc.tile([P, 512], F32, tag="pc",
                                              name=f"o2_{b}_{t}_{n2}")
                                for j in range(ET):
                                    nc.tensor.matmul(
                                        o2[:],
                                        lhsT=yT[j][:, 128 * t:128 * (t + 1)],
                                        rhs=wo_sb[:, j, cs],
                                        start=(j == 0), stop=(j == ET - 1),
                                    )
                                ob = opool.tile([P, 512], BF, tag="ob",
                                                name=f"ob{b}_{t}_{n2}")
                                nc.vector.tensor_add(ob[:], o2[:],
                                                     borep_sb[:, cs])
                                nc.sync.dma_start(
                                    out[b, 128 * t:128 * (t + 1), cs],
                                    ob[:])
                            blocks.append(oblk)
                    return blocks

                blocks = outproj_blocks()
                if b + 1 < BL:
                    # next batch's pair-0 projection first (it gates the
                    # next batch's whole attention stream), then half the
                    # out-proj; the rest carries into b+1's pairs 6-7
                    ensure_qk(b + 1)
                    for blk in proj_subblocks(b + 1, 0):
                        blk()
                    for blk in blocks[:4]:
                        blk()
                    carry = blocks[4:]
                else:
                    for blk in blocks:
                        blk()
            drain(len(pending))

    nc.compile()
    return nc


def _get_nc():
    if "nc" not in _CACHE:
        _CACHE["nc"] = _build()
    return _CACHE["nc"]


def _prep_in_maps(x, Wq, Wk, Wv, Wo, bo):
    bf16 = ml_dtypes.bfloat16
    # [B,T,E] -> [B,E,T] transposed activations
    xT = np.ascontiguousarray(np.asarray(x).transpose(0, 2, 1)).astype(bf16)
    # [H,E,Dh] -> [E, H*Dh] (heads side by side so a 128-col slice = 2 heads)
    wq_pk = np.ascontiguousarray(
        np.asarray(Wq).transpose(1, 0, 2).reshape(E, H * Dh)).astype(bf16)
    wk_pk = np.ascontiguousarray(
        np.asarray(Wk).transpose(1, 0, 2).reshape(E, H * Dh)).astype(bf16)
    wv_pk = np.ascontiguousarray(
        np.asarray(Wv).transpose(1, 0, 2).reshape(E, H * Dh)).astype(bf16)
    wo_b = np.ascontiguousarray(np.asarray(Wo)).astype(bf16)
    borep = np.ascontiguousarray(
        np.broadcast_to(np.asarray(bo, np.float32), (P, E))).astype(bf16)
    ident = np.eye(P, dtype=bf16)
    uneg = (-1e30 * np.tril(np.ones((P, P), np.float32), -1)).astype(bf16)

    in_maps = []
    for c in range(NCORES):
        in_maps.append({
            "xT": xT[BL * c:BL * (c + 1)],
            "wq": wq_pk, "wk": wk_pk, "wv": wv_pk, "wo": wo_b,
            "borep": borep, "ident": ident, "uneg": uneg,
        })
    return in_maps


def run(inputs, trace=False):
    """Returns (full_output [B,T,E] fp32, BassKernelResults)."""
    nc = _get_nc()
    in_maps = _prep_in_maps(**inputs)
    res = run_bass_kernel_spmd(nc, in_maps, core_ids=list(range(NCORES)),
                               trace=trace)
    out = np.concatenate([res.results[c]["out"] for c in range(NCORES)],
                         axis=0).astype(np.float32)
    return out, res


def kernel(x, Wq, Wk, Wv, Wo, bo):
    out, _ = run(dict(x=x, Wq=Wq, Wk=Wk, Wv=Wv, Wo=Wo, bo=bo))
    return out


# revision 29
# speedup vs baseline: 1.0036x; 1.0036x over previous
"""Causal multi-head attention (B=16, T=1024, E=1024, H=16, Dh=64) on 8 TRN2
NeuronCores.

Sharding: data-parallel over batch -- 2 batch elements per core, weights
replicated, no collectives. Host pre-transposes x and packs weights; each core
runs an identical Bass/Tile program on its shard.

Per-core dataflow (all in "transposed" orientation so no on-chip transposes
are ever needed):
  x^T [E,T] (host)   --matmul-->  Q^T,K^T [Dh,T] per head (head-pairs packed
                                  into 128 partitions; 1/sqrt(Dh) folded into
                                  the Q PSUM->SBUF copy)
                     --matmul-->  V [T,Dh] per head (+ ones column)
  S^T[tk,tq] = (K^T tile).T @ Q^T  per key-tile, causal tiles skipped; the
        diagonal 128x128 block gets -1e30 added by a const matmul
        (lhsT=I, rhs=-1e30*tril(,-1)) accumulated into the same PSUM group,
        so exp() lands masked with no vector-engine hop
  P^T = exp(S^T) on ScalarE (scores are O(1): no max subtraction needed)
  O'^T[65,tq] += (V'|1).T @ P^T   -- row 64 accumulates the softmax denom
  Y^T = O'^T[0:64] * bcast(1/denom)
  out[t,E] = Y^T.T @ Wo + bo

Engine assignment (ScalarE is the S->P->O critical hop, keep it exp-only):
  ScalarE: exp
  DVE:     all PSUM->SBUF copies (Q/K scaled copy, V, Y^T), den staging,
           reciprocal_approx_fast, normalize muls
  GpSimd:  partition_broadcast of 1/den (its only fast op; no PSUM access)
  PE:      matmuls incl. the diag-mask const matmul

Scheduling notes (hard-won on HW):
  - engine APs need partition base in {0,32,64,96}; partition_broadcast reads
    physical partition 0 and writes from partition 0 (base-64 dst broken);
    tensor_tensor wants equal bases when both operands are SBUF (a PSUM in0
    at base 0 with SBUF in1/out at base 64 is fine).
  - per-pair qT/kT/yT tiles: Tile deps are whole-tile, per-pair tiles stop
    S matmuls from false-waiting on the next pair's projection copies.
  - HAM unthrottles the PE (1.2 -> 2.4 GHz) only on a fully-busy 3.4us
    window, so each pair's Q/K projection matmuls are interleaved into the
    previous pair's attention stream as dense filler, and 8 of each batch's
    out-projection blocks are carried into the NEXT batch's pairs 6-7
    (which otherwise have no projection filler).
  - normalize chain (recip + bcast + mul) is drip-emitted between i-steps;
    the yT copy + den staging run eagerly so the O' PSUM banks free fast.
"""
import numpy as np
import ml_dtypes

import concourse.bass as bass
import concourse.mybir as mybir
import concourse.tile as tile
from concourse import bacc
from concourse.bass_utils import run_bass_kernel_spmd

B, T, E = 16, 1024, 1024
H, Dh = 16, 64
NCORES = 8
BL = B // NCORES          # batches per core
P = 128                   # partitions
ET = E // P               # 8 tiles along E / token / hd dims
HP = H // 2               # 8 head-pairs
BF = mybir.dt.bfloat16
F32 = mybir.dt.float32
AF = mybir.ActivationFunctionType
ALU = mybir.AluOpType

_CACHE = {}


def _pieces(i):
    """Column pieces of [128*i, 1024) that do not cross the 512 PSUM-bank
    boundary."""
    if i < 4:
        return [(128 * i, 512), (512, 1024)]
    return [(128 * i, 1024)]


def _build():
    nc = bacc.Bacc("TRN2", target_bir_lowering=False, debug=False,
                   num_devices=NCORES)

    xT = nc.dram_tensor("xT", [BL, E, T], BF, kind="ExternalInput").ap()
    wq = nc.dram_tensor("wq", [E, H * Dh], BF, kind="ExternalInput").ap()
    wk = nc.dram_tensor("wk", [E, H * Dh], BF, kind="ExternalInput").ap()
    wv = nc.dram_tensor("wv", [E, H * Dh], BF, kind="ExternalInput").ap()
    wo = nc.dram_tensor("wo", [H * Dh, E], BF, kind="ExternalInput").ap()
    borep = nc.dram_tensor("borep", [P, E], BF, kind="ExternalInput").ap()
    ident = nc.dram_tensor("ident", [P, P], BF, kind="ExternalInput").ap()
    uneg = nc.dram_tensor("uneg", [P, P], BF, kind="ExternalInput").ap()
    out = nc.dram_tensor("out", [BL, T, E], BF,
                         kind="ExternalOutput").ap()

    with tile.TileContext(nc) as tc:
        with (
            tc.tile_pool(name="consts", bufs=1) as cpool,
            tc.tile_pool(name="xp", bufs=1) as xpool,
            tc.tile_pool(name="qk", bufs=1) as qkpool,
            tc.tile_pool(name="vp2", bufs=2) as vpool,
            tc.tile_pool(name="vy", bufs=2) as vypool,
            tc.tile_pool(name="pt", bufs=4) as ptpool,
            tc.tile_pool(name="r1p", bufs=2) as r1pool,
            tc.tile_pool(name="rbp", bufs=2) as rbpool,
            tc.tile_pool(name="ob", bufs=2) as opool,
            tc.tile_pool(name="pso", bufs=2, space="PSUM") as pso,
            tc.tile_pool(name="psc", bufs=4, space="PSUM") as psc,
        ):
            # --- weights / consts; DMA order tracks the startup critical
            # path: V-projection needs Wv halves + xT halves first ---
            wva = cpool.tile([P, ET, 512], BF, tag="wva")
            wvb = cpool.tile([P, ET, 512], BF, tag="wvb")
            wq_sb = cpool.tile([P, ET, H * Dh], BF, tag="wq")
            wk_sb = cpool.tile([P, ET, H * Dh], BF, tag="wk")
            wo_sb = cpool.tile([P, ET, E], BF, tag="wo")
            id_sb = cpool.tile([P, P], BF, tag="ident")
            un_sb = cpool.tile([P, P], BF, tag="uneg")

            wv_r = wv.rearrange("(n p) c -> p n c", p=P)

            xT_tiles = {}
            v_tiles = {}

            def load_x(b, chunked=False):
                """xT halves (tokens 0:512 / 512:1024) for batch b. Chunked
                per E-tile so the first V-proj chain streams behind the DMA
                (Tile tracks subtile deps)."""
                xa = xpool.tile([P, ET, 512], BF, tag="xa", name=f"xa{b}")
                xb = xpool.tile([P, ET, 512], BF, tag="xb", name=f"xb{b}")
                xr = xT[b].rearrange("(n p) c -> p n c", p=P)
                if chunked:
                    for i in range(ET):
                        nc.sync.dma_start(wva[:, i, :], wv_r[:, i, 0:512])
                        nc.sync.dma_start(xa[:, i, :], xr[:, i, 0:512])
                    for i in range(ET):
                        nc.sync.dma_start(wvb[:, i, :], wv_r[:, i, 512:1024])
                        nc.sync.dma_start(xb[:, i, :], xr[:, i, 512:1024])
                else:
                    nc.sync.dma_start(xa[:], xr[:, :, 0:512])
                    nc.sync.dma_start(xb[:], xr[:, :, 512:1024])
                xT_tiles[b] = (xa, xb)
                return xa, xb, xr

            def load_v_tile(b):
                v_tiles[b] = vpool.tile([P, ET, H, Dh + 1], BF, tag="v",
                                        name=f"v{b}")
                nc.vector.memset(v_tiles[b][:, :, :, Dh], 1.0)

            def x_ap(b, t, cols):
                """lhsT slice of x^T for t-tile t (128 tokens) col range."""
                xa, xb = xT_tiles[b]
                if t < 4:
                    return xa[:, cols, 128 * t:128 * (t + 1)]
                return xb[:, cols, 128 * (t - 4):128 * (t - 3)]

            def v_blocks(b, order=None):
                """V-projection blocks; block (t, n2) computes v[:, t,
                8*n2:8*(n2+1), :]. Order tuned so the first blocks only
                need wva + xa."""
                if order is None:
                    order = [(t, n2) for n2 in range(2) for t in range(ET)]
                blocks = []
                for (t, n2) in order:
                    def vblk(t=t, n2=n2, b=b):
                        w_sb = wva if n2 == 0 else wvb
                        vp = psc.tile([P, 512], F32, tag="pc",
                                      name=f"vp{b}_{t}_{n2}")
                        for i in range(ET):
                            nc.tensor.matmul(
                                vp[:],
                                lhsT=x_ap(b, t, i),
                                rhs=w_sb[:, i, :],
                                start=(i == 0), stop=(i == ET - 1),
                            )
                        nc.scalar.activation(
                            v_tiles[b][:, t, 8 * n2:8 * (n2 + 1), 0:Dh],
                            vp[:].rearrange("p (h d) -> p h d", d=Dh),
                            AF.Copy)
                    blocks.append(vblk)
                return blocks

            # batch 0 startup: interleaved per-E-tile wv/x chunks
            load_x(0, chunked=True)
            load_v_tile(0)
            order0 = ([(t, 0) for t in range(4)] + [(t, 1) for t in range(4)]
                      + [(t, 0) for t in range(4, ET)]
                      + [(t, 1) for t in range(4, ET)])
            blocks0 = v_blocks(0, order0)
            for blk in blocks0[:8]:
                blk()
            nc.sync.dma_start(wq_sb[:], wq.rearrange("(n p) c -> p n c", p=P))
            nc.sync.dma_start(wk_sb[:], wk.rearrange("(n p) c -> p n c", p=P))
            nc.sync.dma_start(id_sb[:], ident)
            nc.sync.dma_start(un_sb[:], uneg)
            for blk in blocks0[8:]:
                blk()
            nc.sync.dma_start(wo_sb[:], wo.rearrange("(n p) c -> p n c", p=P))
            borep_sb = cpool.tile([P, E], BF, tag="bo")
            nc.sync.dma_start(borep_sb[:], borep)

            pending = []

            def drain(n):
                for _ in range(min(n, len(pending))):
                    pending.pop(0)()

            carry = []  # prev batch's deferred out-proj blocks
            qT_all, kT_all = {}, {}

            def ensure_qk(b):
                if b in qT_all:
                    return
                qT_all[b] = [qkpool.tile([P, T], BF, tag=f"q{pp}",
                                         name=f"q{b}_{pp}")
                             for pp in range(HP)]
                kT_all[b] = [qkpool.tile([P, T], BF, tag=f"k{pp}",
                                         name=f"k{b}_{pp}")
                             for pp in range(HP)]

            def proj_subblocks(b, pp):
                blocks = []
                for (w_sb, dstl, scale) in ((wq_sb, qT_all[b], 0.125),
                                            (wk_sb, kT_all[b], None)):
                    for n2 in range(2):
                        def blk(w_sb=w_sb, dstl=dstl, scale=scale,
                                n2=n2, pp=pp, b=b):
                            cs = slice(512 * n2, 512 * (n2 + 1))
                            xh = xT_tiles[b][n2]
                            pj = psc.tile([P, 512], F32, tag="pc",
                                          name=f"pj{b}_{pp}_{n2}")
                            for i in range(ET):
                                nc.tensor.matmul(
                                    pj[:],
                                    lhsT=w_sb[:, i,
                                              128 * pp:128 * (pp + 1)],
                                    rhs=xh[:, i, :],
                                    start=(i == 0), stop=(i == ET - 1),
                                )
                            if scale is None:
                                nc.vector.tensor_copy(dstl[pp][:, cs],
                                                      pj[:])
                            else:
                                nc.vector.tensor_scalar_mul(
                                    dstl[pp][:, cs], pj[:], scale)
                        blocks.append(blk)
                return blocks

            for b in range(BL):
                v_sb = v_tiles[b]
                ensure_qk(b)
                qT, kT = qT_all[b], kT_all[b]

                if b == 0:
                    for blk in proj_subblocks(0, 0):
                        blk()
                # (b>0: pair-0 projection was emitted at the end of b-1)

                # ---- per-pair attention ----
                yT = [vypool.tile([P, T], BF, tag=f"y{pp}", name=f"y{b}_{pp}")
                      for pp in range(HP)]

                for hp in range(HP):
                    if hp + 1 < HP:
                        pending[0:0] = proj_subblocks(b, hp + 1)
                    if hp == 6:
                        if b + 1 < BL:
                            def ldnext(b=b):
                                load_x(b + 1)
                                load_v_tile(b + 1)
                            pending.append(ldnext)
                            pending.extend(v_blocks(b + 1))
                        else:
                            pending.extend(carry)
                            carry = []
                    ops = [pso.tile([P, 1024], F32, tag="op",
                                    name=f"op{b}_{hp}_{s}") for s in range(2)]
                    eager = (hp == HP - 1)
                    for i in range(ET):
                        pts = []
                        for sub in (0, 1):
                            pt = ptpool.tile([P, 1024], BF, tag="pt",
                                             name=f"pt{b}_{hp}_{i}_{sub}")
                            pts.append(pt)
                        for pidx, (a0, a1) in enumerate(_pieces(i)):
                            w = a1 - a0
                            sps = []
                            for sub in (0, 1):
                                po = Dh * sub
                                sp_ = psc.tile(
                                    [P, 512], F32, tag="pc",
                                    name=f"sp{b}_{hp}_{i}_{sub}_{a0}")
                                nc.tensor.matmul(
                                    sp_[:, 0:w],
                                    lhsT=kT[hp][po:po + Dh,
                                                128 * i:128 * (i + 1)],
                                    rhs=qT[hp][po:po + Dh, a0:a1],
                                    start=True, stop=(pidx != 0),
                                    skip_group_check=True,
                                )
                                sps.append(sp_)
                            if pidx == 0:
                                # diag 128x128 block: add -1e30 upper mask
                                for sub in (0, 1):
                                    nc.tensor.matmul(
                                        sps[sub][:, 0:P],
                                        lhsT=id_sb[:],
                                        rhs=un_sb[:],
                                        start=False, stop=True,
                                        skip_group_check=True,
                                    )
                            for sub in (0, 1):
                                nc.scalar.activation(pts[sub][:, a0:a1],
                                                     sps[sub][:, 0:w], AF.Exp)
                        for sub in (0, 1):
                            h = 2 * hp + sub
                            for (a0, a1) in _pieces(i):
                                nc.tensor.matmul(
                                    ops[sub][0:Dh + 1, a0:a1],
                                    lhsT=v_sb[:, i, h, :],
                                    rhs=pts[sub][:, a0:a1],
                                    start=(i == 0), stop=(i == ET - 1),
                                    skip_group_check=True,
                                )
                            if i == ET - 1:
                                po = Dh * sub
                                # eager: copy Y^T (DVE) + stage denom
                                # (ACT) so the O' PSUM banks free fast; the
                                # stage fills ScalarE's pair-boundary exp
                                # lull instead of delaying the next pair's
                                # exps behind a 1.1us yT copy
                                r1 = r1pool.tile([1, T], F32, tag="r1",
                                                 name=f"r1_{b}_{h}")
                                nc.vector.tensor_copy(
                                    yT[hp][po:po + Dh, :],
                                    ops[sub][0:Dh, :])
                                nc.scalar.activation(
                                    r1[0:1, :], ops[sub][Dh:Dh + 1, :],
                                    AF.Copy)

                                def recip(r1=r1):
                                    nc.vector.reciprocal_approx_fast(
                                        r1[0:1, :], r1[0:1, :])
                                holder = {}

                                def bcast(r1=r1, holder=holder, b=b, h=h):
                                    rb = rbpool.tile([P, T], F32, tag="rb",
                                                     name=f"rb_{b}_{h}")
                                    nc.gpsimd.partition_broadcast(rb[:],
                                                                  r1[0:1, :])
                                    holder['rb'] = rb

                                def nmul(hp=hp, po=po, holder=holder):
                                    ap = yT[hp]
                                    nc.vector.tensor_mul(
                                        ap[po:po + Dh, :], ap[po:po + Dh, :],
                                        holder['rb'][po:po + Dh, :])
                                if eager:
                                    recip(); bcast(); nmul()
                                else:
                                    pending.extend([recip, bcast, nmul])
                        drain(2)

                # emission order IS dependency order under Tile's tracer:
                # all normalize muls must be emitted before out-proj reads yT
                drain(len(pending))

                # ---- output projection + bias ----
                def outproj_blocks(b=b, yT=yT):
                    blocks = []
                    for t in range(ET):
                        for n2 in range(2):
                            def oblk(t=t, n2=n2, b=b, yT=yT):
                                cs = slice(512 * n2, 512 * (n2 + 1))
                                o2 = psc.tile([P, 512], F32, tag="pc",
                                              name=f"o2_{b}_{t}_{n2}")
                                for j in range(ET):
                                    nc.tensor.matmul(
                                        o2[:],
                                        lhsT=yT[j][:, 128 * t:128 * (t + 1)],
                                        rhs=wo_sb[:, j, cs],
                                        start=(j == 0), stop=(j == ET - 1),
                                    )
                                ob = opool.tile([P, 512], BF, tag="ob",
                                                name=f"ob{b}_{t}_{n2}")
                                nc.vector.tensor_add(ob[:], o2[:],
                                                     borep_sb[:, cs])
                                nc.sync.dma_start(
                                    out[b, 128 * t:128 * (t + 1), cs],
                                    ob[:])
                            blocks.append(oblk)
                    return blocks

                blocks = outproj_blocks()
                if b + 1 < BL:
                    # next batch's pair-0 projection first (it gates the
                    # next batch's whole attention stream), then half the
                    # out-proj; the rest carries into b+1's pairs 6-7
                    ensure_qk(b + 1)
                    for blk in proj_subblocks(b + 1, 0):
                        blk()
                    for blk in blocks[:8]:
                        blk()
                    carry = blocks[8:]
                else:
                    for blk in blocks:
                        blk()
            drain(len(pending))

    nc.compile()
    return nc


def _get_nc():
    if "nc" not in _CACHE:
        _CACHE["nc"] = _build()
    return _CACHE["nc"]


def _prep_in_maps(x, Wq, Wk, Wv, Wo, bo):
    bf16 = ml_dtypes.bfloat16
    # [B,T,E] -> [B,E,T] transposed activations
    xT = np.ascontiguousarray(np.asarray(x).transpose(0, 2, 1)).astype(bf16)
    # [H,E,Dh] -> [E, H*Dh] (heads side by side so a 128-col slice = 2 heads)
    wq_pk = np.ascontiguousarray(
        np.asarray(Wq).transpose(1, 0, 2).reshape(E, H * Dh)).astype(bf16)
    wk_pk = np.ascontiguousarray(
        np.asarray(Wk).transpose(1, 0, 2).reshape(E, H * Dh)).astype(bf16)
    wv_pk = np.ascontiguousarray(
        np.asarray(Wv).transpose(1, 0, 2).reshape(E, H * Dh)).astype(bf16)
    wo_b = np.ascontiguousarray(np.asarray(Wo)).astype(bf16)
    borep = np.ascontiguousarray(
        np.broadcast_to(np.asarray(bo, np.float32), (P, E))).astype(bf16)
    ident = np.eye(P, dtype=bf16)
    uneg = (-1e30 * np.tril(np.ones((P, P), np.float32), -1)).astype(bf16)

    in_maps = []
    for c in range(NCORES):
        in_maps.append({
            "xT": xT[BL * c:BL * (c + 1)],
            "wq": wq_pk, "wk": wk_pk, "wv": wv_pk, "wo": wo_b,
            "borep": borep, "ident": ident, "uneg": uneg,
        })
    return in_maps


def run(inputs, trace=False):
    """Returns (full_output [B,T,E] fp32, BassKernelResults)."""
    nc = _get_nc()
    in_maps = _prep_in_maps(**inputs)
    res = run_bass_kernel_spmd(nc, in_maps, core_ids=list(range(NCORES)),
                               trace=trace)
    out = np.concatenate([res.results[c]["out"] for c in range(NCORES)],
                         axis=0).astype(np.float32)
    return out, res


def kernel(x, Wq, Wk, Wv, Wo, bo):
    out, _ = run(dict(x=x, Wq=Wq, Wk=Wk, Wv=Wv, Wo=Wo, bo=bo))
    return out


# revision 30
# speedup vs baseline: 1.0078x; 1.0042x over previous
"""Causal multi-head attention (B=16, T=1024, E=1024, H=16, Dh=64) on 8 TRN2
NeuronCores.

Sharding: data-parallel over batch -- 2 batch elements per core, weights
replicated, no collectives. Host pre-transposes x and packs weights; each core
runs an identical Bass/Tile program on its shard.

Per-core dataflow (all in "transposed" orientation so no on-chip transposes
are ever needed):
  x^T [E,T] (host)   --matmul-->  Q^T,K^T [Dh,T] per head (head-pairs packed
                                  into 128 partitions; 1/sqrt(Dh) folded into
                                  the Q PSUM->SBUF copy)
                     --matmul-->  V [T,Dh] per head (+ ones column)
  S^T[tk,tq] = (K^T tile).T @ Q^T  per key-tile, causal tiles skipped; the
        diagonal 128x128 block gets -1e30 added by a const matmul
        (lhsT=I, rhs=-1e30*tril(,-1)) accumulated into the same PSUM group,
        so exp() lands masked with no vector-engine hop
  P^T = exp(S^T) on ScalarE (scores are O(1): no max subtraction needed)
  O'^T[65,tq] += (V'|1).T @ P^T   -- row 64 accumulates the softmax denom
  Y^T = O'^T[0:64] * bcast(1/denom)
  out[t,E] = Y^T.T @ Wo + bo

Engine assignment (ScalarE is the S->P->O critical hop, keep it exp-only):
  ScalarE: exp
  DVE:     all PSUM->SBUF copies (Q/K scaled copy, V, Y^T), den staging,
           reciprocal_approx_fast, normalize muls
  GpSimd:  partition_broadcast of 1/den (its only fast op; no PSUM access)
  PE:      matmuls incl. the diag-mask const matmul

Scheduling notes (hard-won on HW):
  - engine APs need partition base in {0,32,64,96}; partition_broadcast reads
    physical partition 0 and writes from partition 0 (base-64 dst broken);
    tensor_tensor wants equal bases when both operands are SBUF (a PSUM in0
    at base 0 with SBUF in1/out at base 64 is fine).
  - per-pair qT/kT/yT tiles: Tile deps are whole-tile, per-pair tiles stop
    S matmuls from false-waiting on the next pair's projection copies.
  - HAM unthrottles the PE (1.2 -> 2.4 GHz) only on a fully-busy 3.4us
    window, so each pair's Q/K projection matmuls are interleaved into the
    previous pair's attention stream as dense filler, and 8 of each batch's
    out-projection blocks are carried into the NEXT batch's pairs 6-7
    (which otherwise have no projection filler).
  - normalize chain (recip + bcast + mul) is drip-emitted between i-steps;
    the yT copy + den staging run eagerly so the O' PSUM banks free fast.
"""
import numpy as np
import ml_dtypes

import concourse.bass as bass
import concourse.mybir as mybir
import concourse.tile as tile
from concourse import bacc
from concourse.bass_utils import run_bass_kernel_spmd

B, T, E = 16, 1024, 1024
H, Dh = 16, 64
NCORES = 8
BL = B // NCORES          # batches per core
P = 128                   # partitions
ET = E // P               # 8 tiles along E / token / hd dims
HP = H // 2               # 8 head-pairs
BF = mybir.dt.bfloat16
F32 = mybir.dt.float32
AF = mybir.ActivationFunctionType
ALU = mybir.AluOpType

_CACHE = {}


def _pieces(i):
    """Column pieces of [128*i, 1024) that do not cross the 512 PSUM-bank
    boundary."""
    if i < 4:
        return [(128 * i, 512), (512, 1024)]
    return [(128 * i, 1024)]


def _build():
    nc = bacc.Bacc("TRN2", target_bir_lowering=False, debug=False,
                   num_devices=NCORES)

    xT = nc.dram_tensor("xT", [BL, E, T], BF, kind="ExternalInput").ap()
    wq = nc.dram_tensor("wq", [E, H * Dh], BF, kind="ExternalInput").ap()
    wk = nc.dram_tensor("wk", [E, H * Dh], BF, kind="ExternalInput").ap()
    wv = nc.dram_tensor("wv", [E, H * Dh], BF, kind="ExternalInput").ap()
    wo = nc.dram_tensor("wo", [H * Dh, E], BF, kind="ExternalInput").ap()
    borep = nc.dram_tensor("borep", [P, E], BF, kind="ExternalInput").ap()
    ident = nc.dram_tensor("ident", [P, P], BF, kind="ExternalInput").ap()
    uneg = nc.dram_tensor("uneg", [P, P], BF, kind="ExternalInput").ap()
    out = nc.dram_tensor("out", [BL, T, E], BF,
                         kind="ExternalOutput").ap()

    with tile.TileContext(nc) as tc:
        with (
            tc.tile_pool(name="consts", bufs=1) as cpool,
            tc.tile_pool(name="xp", bufs=1) as xpool,
            tc.tile_pool(name="qk", bufs=1) as qkpool,
            tc.tile_pool(name="vp2", bufs=2) as vpool,
            tc.tile_pool(name="vy", bufs=2) as vypool,
            tc.tile_pool(name="pt", bufs=4) as ptpool,
            tc.tile_pool(name="r1p", bufs=2) as r1pool,
            tc.tile_pool(name="rbp", bufs=2) as rbpool,
            tc.tile_pool(name="ob", bufs=2) as opool,
            tc.tile_pool(name="pso", bufs=2, space="PSUM") as pso,
            tc.tile_pool(name="psc", bufs=4, space="PSUM") as psc,
        ):
            # --- weights / consts; DMA order tracks the startup critical
            # path: V-projection needs Wv halves + xT halves first ---
            wva = cpool.tile([P, ET, 512], BF, tag="wva")
            wvb = cpool.tile([P, ET, 512], BF, tag="wvb")
            wq_sb = cpool.tile([P, ET, H * Dh], BF, tag="wq")
            wk_sb = cpool.tile([P, ET, H * Dh], BF, tag="wk")
            wo_sb = cpool.tile([P, ET, E], BF, tag="wo")
            id_sb = cpool.tile([P, P], BF, tag="ident")
            un_sb = cpool.tile([P, P], BF, tag="uneg")

            wv_r = wv.rearrange("(n p) c -> p n c", p=P)

            xT_tiles = {}
            v_tiles = {}

            def load_x(b, chunked=False):
                """xT halves (tokens 0:512 / 512:1024) for batch b. Chunked
                per E-tile so the first V-proj chain streams behind the DMA
                (Tile tracks subtile deps)."""
                xa = xpool.tile([P, ET, 512], BF, tag="xa", name=f"xa{b}")
                xb = xpool.tile([P, ET, 512], BF, tag="xb", name=f"xb{b}")
                xr = xT[b].rearrange("(n p) c -> p n c", p=P)
                if chunked:
                    # arrival order = consumption order: the V blocks run
                    # (t0-3,n2=0) on wva+xa, then (t0-3,n2=1) on wvb, then
                    # (t4-7,*) on xb -- interleaving wvb with xb would feed
                    # the n2=1 blocks at half DMA rate
                    for i in range(ET):
                        nc.sync.dma_start(wva[:, i, :], wv_r[:, i, 0:512])
                        nc.sync.dma_start(xa[:, i, :], xr[:, i, 0:512])
                    for i in range(ET):
                        nc.sync.dma_start(wvb[:, i, :], wv_r[:, i, 512:1024])
                    for i in range(ET):
                        nc.sync.dma_start(xb[:, i, :], xr[:, i, 512:1024])
                else:
                    nc.sync.dma_start(xa[:], xr[:, :, 0:512])
                    nc.sync.dma_start(xb[:], xr[:, :, 512:1024])
                xT_tiles[b] = (xa, xb)
                return xa, xb, xr

            def load_v_tile(b):
                v_tiles[b] = vpool.tile([P, ET, H, Dh + 1], BF, tag="v",
                                        name=f"v{b}")
                nc.vector.memset(v_tiles[b][:, :, :, Dh], 1.0)

            def x_ap(b, t, cols):
                """lhsT slice of x^T for t-tile t (128 tokens) col range."""
                xa, xb = xT_tiles[b]
                if t < 4:
                    return xa[:, cols, 128 * t:128 * (t + 1)]
                return xb[:, cols, 128 * (t - 4):128 * (t - 3)]

            def v_blocks(b, order=None):
                """V-projection blocks; block (t, n2) computes v[:, t,
                8*n2:8*(n2+1), :]. Order tuned so the first blocks only
                need wva + xa."""
                if order is None:
                    order = [(t, n2) for n2 in range(2) for t in range(ET)]
                blocks = []
                for (t, n2) in order:
                    def vblk(t=t, n2=n2, b=b):
                        w_sb = wva if n2 == 0 else wvb
                        vp = psc.tile([P, 512], F32, tag="pc",
                                      name=f"vp{b}_{t}_{n2}")
                        for i in range(ET):
                            nc.tensor.matmul(
                                vp[:],
                                lhsT=x_ap(b, t, i),
                                rhs=w_sb[:, i, :],
                                start=(i == 0), stop=(i == ET - 1),
                            )
                        nc.scalar.activation(
                            v_tiles[b][:, t, 8 * n2:8 * (n2 + 1), 0:Dh],
                            vp[:].rearrange("p (h d) -> p h d", d=Dh),
                            AF.Copy)
                    blocks.append(vblk)
                return blocks

            # batch 0 startup: interleaved per-E-tile wv/x chunks
            load_x(0, chunked=True)
            load_v_tile(0)
            order0 = ([(t, 0) for t in range(4)] + [(t, 1) for t in range(4)]
                      + [(t, 0) for t in range(4, ET)]
                      + [(t, 1) for t in range(4, ET)])
            blocks0 = v_blocks(0, order0)
            for blk in blocks0[:8]:
                blk()
            nc.sync.dma_start(wq_sb[:], wq.rearrange("(n p) c -> p n c", p=P))
            nc.sync.dma_start(wk_sb[:], wk.rearrange("(n p) c -> p n c", p=P))
            nc.sync.dma_start(id_sb[:], ident)
            nc.sync.dma_start(un_sb[:], uneg)
            for blk in blocks0[8:]:
                blk()
            nc.sync.dma_start(wo_sb[:], wo.rearrange("(n p) c -> p n c", p=P))
            borep_sb = cpool.tile([P, E], BF, tag="bo")
            nc.sync.dma_start(borep_sb[:], borep)

            pending = []

            def drain(n):
                for _ in range(min(n, len(pending))):
                    pending.pop(0)()

            carry = []  # prev batch's deferred out-proj blocks
            qT_all, kT_all = {}, {}

            def ensure_qk(b):
                if b in qT_all:
                    return
                qT_all[b] = [qkpool.tile([P, T], BF, tag=f"q{pp}",
                                         name=f"q{b}_{pp}")
                             for pp in range(HP)]
                kT_all[b] = [qkpool.tile([P, T], BF, tag=f"k{pp}",
                                         name=f"k{b}_{pp}")
                             for pp in range(HP)]

            def proj_subblocks(b, pp):
                blocks = []
                for (w_sb, dstl, scale) in ((wq_sb, qT_all[b], 0.125),
                                            (wk_sb, kT_all[b], None)):
                    for n2 in range(2):
                        def blk(w_sb=w_sb, dstl=dstl, scale=scale,
                                n2=n2, pp=pp, b=b):
                            cs = slice(512 * n2, 512 * (n2 + 1))
                            xh = xT_tiles[b][n2]
                            pj = psc.tile([P, 512], F32, tag="pc",
                                          name=f"pj{b}_{pp}_{n2}")
                            for i in range(ET):
                                nc.tensor.matmul(
                                    pj[:],
                                    lhsT=w_sb[:, i,
                                              128 * pp:128 * (pp + 1)],
                                    rhs=xh[:, i, :],
                                    start=(i == 0), stop=(i == ET - 1),
                                )
                            if scale is None:
                                nc.vector.tensor_copy(dstl[pp][:, cs],
                                                      pj[:])
                            else:
                                nc.vector.tensor_scalar_mul(
                                    dstl[pp][:, cs], pj[:], scale)
                        blocks.append(blk)
                return blocks

            for b in range(BL):
                v_sb = v_tiles[b]
                ensure_qk(b)
                qT, kT = qT_all[b], kT_all[b]

                if b == 0:
                    for blk in proj_subblocks(0, 0):
                        blk()
                # (b>0: pair-0 projection was emitted at the end of b-1)

                # ---- per-pair attention ----
                yT = [vypool.tile([P, T], BF, tag=f"y{pp}", name=f"y{b}_{pp}")
                      for pp in range(HP)]

                for hp in range(HP):
                    if hp + 1 < HP:
                        pending[0:0] = proj_subblocks(b, hp + 1)
                    if hp == 6:
                        if b + 1 < BL:
                            def ldnext(b=b):
                                load_x(b + 1)
                                load_v_tile(b + 1)
                            pending.append(ldnext)
                            pending.extend(v_blocks(b + 1))
                        else:
                            pending.extend(carry)
                            carry = []
                    ops = [pso.tile([P, 1024], F32, tag="op",
                                    name=f"op{b}_{hp}_{s}") for s in range(2)]
                    eager = (hp == HP - 1)
                    for i in range(ET):
                        pts = []
                        for sub in (0, 1):
                            pt = ptpool.tile([P, 1024], BF, tag="pt",
                                             name=f"pt{b}_{hp}_{i}_{sub}")
                            pts.append(pt)
                        for pidx, (a0, a1) in enumerate(_pieces(i)):
                            w = a1 - a0
                            sps = []
                            for sub in (0, 1):
                                po = Dh * sub
                                sp_ = psc.tile(
                                    [P, 512], F32, tag="pc",
                                    name=f"sp{b}_{hp}_{i}_{sub}_{a0}")
                                nc.tensor.matmul(
                                    sp_[:, 0:w],
                                    lhsT=kT[hp][po:po + Dh,
                                                128 * i:128 * (i + 1)],
                                    rhs=qT[hp][po:po + Dh, a0:a1],
                                    start=True, stop=(pidx != 0),
                                    skip_group_check=True,
                                )
                                sps.append(sp_)
                            if pidx == 0:
                                # diag 128x128 block: add -1e30 upper mask
                                for sub in (0, 1):
                                    nc.tensor.matmul(
                                        sps[sub][:, 0:P],
                                        lhsT=id_sb[:],
                                        rhs=un_sb[:],
                                        start=False, stop=True,
                                        skip_group_check=True,
                                    )
                            for sub in (0, 1):
                                nc.scalar.activation(pts[sub][:, a0:a1],
                                                     sps[sub][:, 0:w], AF.Exp)
                        for sub in (0, 1):
                            h = 2 * hp + sub
                            for (a0, a1) in _pieces(i):
                                nc.tensor.matmul(
                                    ops[sub][0:Dh + 1, a0:a1],
                                    lhsT=v_sb[:, i, h, :],
                                    rhs=pts[sub][:, a0:a1],
                                    start=(i == 0), stop=(i == ET - 1),
                                    skip_group_check=True,
                                )
                            if i == ET - 1:
                                po = Dh * sub
                                # eager: copy Y^T (DVE) + stage denom
                                # (ACT) so the O' PSUM banks free fast; the
                                # stage fills ScalarE's pair-boundary exp
                                # lull instead of delaying the next pair's
                                # exps behind a 1.1us yT copy
                                r1 = r1pool.tile([1, T], F32, tag="r1",
                                                 name=f"r1_{b}_{h}")
                                nc.vector.tensor_copy(
                                    yT[hp][po:po + Dh, :],
                                    ops[sub][0:Dh, :])
                                nc.scalar.activation(
                                    r1[0:1, :], ops[sub][Dh:Dh + 1, :],
                                    AF.Copy)

                                def recip(r1=r1):
                                    nc.vector.reciprocal_approx_fast(
                                        r1[0:1, :], r1[0:1, :])
                                holder = {}

                                def bcast(r1=r1, holder=holder, b=b, h=h):
                                    rb = rbpool.tile([P, T], F32, tag="rb",
                                                     name=f"rb_{b}_{h}")
                                    nc.gpsimd.partition_broadcast(rb[:],
                                                                  r1[0:1, :])
                                    holder['rb'] = rb

                                def nmul(hp=hp, po=po, holder=holder):
                                    ap = yT[hp]
                                    nc.vector.tensor_mul(
                                        ap[po:po + Dh, :], ap[po:po + Dh, :],
                                        holder['rb'][po:po + Dh, :])
                                if eager:
                                    recip(); bcast(); nmul()
                                else:
                                    pending.extend([recip, bcast, nmul])
                        drain(2)

                # emission order IS dependency order under Tile's tracer:
                # all normalize muls must be emitted before out-proj reads yT
                drain(len(pending))

                # ---- output projection + bias ----
                def outproj_blocks(b=b, yT=yT):
                    blocks = []
                    for t in range(ET):
                        for n2 in range(2):
                            def oblk(t=t, n2=n2, b=b, yT=yT):
                                cs = slice(512 * n2, 512 * (n2 + 1))
                                o2 = psc.tile([P, 512], F32, tag="pc",
                                              name=f"o2_{b}_{t}_{n2}")
                                for j in range(ET):
                                    nc.tensor.matmul(
                                        o2[:],
                                        lhsT=yT[j][:, 128 * t:128 * (t + 1)],
                                        rhs=wo_sb[:, j, cs],
                                        start=(j == 0), stop=(j == ET - 1),
                                    )
                                ob = opool.tile([P, 512], BF, tag="ob",
                                                name=f"ob{b}_{t}_{n2}")
                                nc.vector.tensor_add(ob[:], o2[:],
                                                     borep_sb[:, cs])
                                nc.sync.dma_start(
                                    out[b, 128 * t:128 * (t + 1), cs],
                                    ob[:])
                            blocks.append(oblk)
                    return blocks

                blocks = outproj_blocks()
                if b + 1 < BL:
                    # next batch's pair-0 projection first (it gates the
                    # next batch's whole attention stream), then half the
                    # out-proj; the rest carries into b+1's pairs 6-7
                    ensure_qk(b + 1)
                    for blk in proj_subblocks(b + 1, 0):
                        blk()
                    for blk in blocks[:8]:
                        blk()
                    carry = blocks[8:]
                else:
                    for blk in blocks:
                        blk()
            drain(len(pending))

    nc.compile()
    return nc


def _get_nc():
    if "nc" not in _CACHE:
        _CACHE["nc"] = _build()
    return _CACHE["nc"]


def _prep_in_maps(x, Wq, Wk, Wv, Wo, bo):
    bf16 = ml_dtypes.bfloat16
    # [B,T,E] -> [B,E,T] transposed activations
    xT = np.ascontiguousarray(np.asarray(x).transpose(0, 2, 1)).astype(bf16)
    # [H,E,Dh] -> [E, H*Dh] (heads side by side so a 128-col slice = 2 heads)
    wq_pk = np.ascontiguousarray(
        np.asarray(Wq).transpose(1, 0, 2).reshape(E, H * Dh)).astype(bf16)
    wk_pk = np.ascontiguousarray(
        np.asarray(Wk).transpose(1, 0, 2).reshape(E, H * Dh)).astype(bf16)
    wv_pk = np.ascontiguousarray(
        np.asarray(Wv).transpose(1, 0, 2).reshape(E, H * Dh)).astype(bf16)
    wo_b = np.ascontiguousarray(np.asarray(Wo)).astype(bf16)
    borep = np.ascontiguousarray(
        np.broadcast_to(np.asarray(bo, np.float32), (P, E))).astype(bf16)
    ident = np.eye(P, dtype=bf16)
    uneg = (-1e30 * np.tril(np.ones((P, P), np.float32), -1)).astype(bf16)

    in_maps = []
    for c in range(NCORES):
        in_maps.append({
            "xT": xT[BL * c:BL * (c + 1)],
            "wq": wq_pk, "wk": wk_pk, "wv": wv_pk, "wo": wo_b,
            "borep": borep, "ident": ident, "uneg": uneg,
        })
    return in_maps


def run(inputs, trace=False):
    """Returns (full_output [B,T,E] fp32, BassKernelResults)."""
    nc = _get_nc()
    in_maps = _prep_in_maps(**inputs)
    res = run_bass_kernel_spmd(nc, in_maps, core_ids=list(range(NCORES)),
                               trace=trace)
    out = np.concatenate([res.results[c]["out"] for c in range(NCORES)],
                         axis=0).astype(np.float32)
    return out, res


def kernel(x, Wq, Wk, Wv, Wo, bo):
    out, _ = run(dict(x=x, Wq=Wq, Wk=Wk, Wv=Wv, Wo=Wo, bo=bo))
    return out
